# revision 14
# baseline (speedup 1.0000x reference)
"""CPC model (conv encoder + GRU + InfoNCE loss) on 8 TRN2 NeuronCores.

Strategy (validated in numpy prototype):
 - Data-parallel over batch: each core owns 8 of 64 sequences (72 images).
 - Conv encoder runs per image-pair as bf16 matmuls (f32 PSUM accum):
     conv1 5x5s2 via host im2col (K=75 padded to 128), resblock 3x3 via
     9-tap shifted matmuls over a zero-padded [18,18] activation.
 - Global avg pool -> z in f32; GRU (4 steps), W_k preds, scores and the
   double-exp log-softmax all in f32.
 - One AllGather (f32, 80KB/core) of the ztk timesteps; each core scores its
   8 anchors against all 64 candidates, emits [8, 5] diag log-softmax values
   and [8, 5] argmax indices; host does the final mean / accuracy.
"""
import os
import sys

import numpy as np
import ml_dtypes

for _p in ("/opt/trn_rl_repo", "/root/.axon_site/_ro/trn_rl_repo"):
    if os.path.isdir(_p) and _p not in sys.path:
        sys.path.insert(0, _p)

import concourse.bacc as bacc  # noqa: E402
import concourse.bass as bass  # noqa: E402
import concourse.mybir as mybir  # noqa: E402
import concourse.tile as tile  # noqa: E402
from concourse.bass_utils import run_bass_kernel_spmd  # noqa: E402

F32 = np.float32
BF16 = ml_dtypes.bfloat16
DT = mybir.dt

B, T, C = 64, 9, 3
DIM, HALF, HID, R, K = 512, 256, 256, 2, 5
TCTX = 4
NCORES = 8
NB = B // NCORES           # 8
NIMG = NB * T              # 72
NPIX = 256                 # 16*16
NPAIR = NIMG // 2          # 36
ALU = mybir.AluOpType
ACTF = mybir.ActivationFunctionType


def build_kernel(stage='full'):
    # stage: 'conv' (encoder+pool only), 'gru' (+GRU/preds), 'cc' (+AllGather),
    #        'full' (+scores/loss)
    nc = bacc.Bacc("TRN2", target_bir_lowering=False, debug=False,
                   num_devices=NCORES)

    def din(name, shape, dt):
        return nc.dram_tensor(name, shape, dt, kind="ExternalInput")

    xcol_d = din("xcol", [NPAIR, 128, 512], DT.bfloat16)
    w1T_d = din("w1T", [128, DIM], DT.bfloat16)
    r1T_d = din("r1T", [128, R, 4, HALF], DT.bfloat16)
    w2T_d = din("w2T", [128, R, 9, 2, HALF], DT.bfloat16)
    r3T_d = din("r3T", [128, R, 2, DIM], DT.bfloat16)
    encb_d = din("encb", [128, 4], DT.float32)
    b1_d = din("b1", [128, R, 2], DT.float32)
    b2_d = din("b2", [128, R, 2], DT.float32)
    b3_d = din("b3", [128, R, 4], DT.float32)
    gihT_d = din("gihT", [128, 4, 3 * HID], DT.float32)
    ghhT_d = din("ghhT", [128, 2, 3 * HID], DT.float32)
    gbih_d = din("gbih", [NB, 3 * HID], DT.float32)   # host-broadcast over batch
    gbhh_d = din("gbhh", [NB, 3 * HID], DT.float32)
    wkT_d = din("wkT", [128, K, 2, DIM], DT.float32)
    wkb_d = din("wkb", [128, K, 4], DT.float32)
    mask_d = din("mask", [NB, K * B], DT.float32)     # one-hot diag per (i,k)
    ident_d = din("ident8", [NB, NB], DT.float32)
    iota_d = din("iota64", [NB, B], DT.float32)   # 0..63 per row

    out_d = nc.dram_tensor("out", [NB, 2 * K], DT.float32, kind="ExternalOutput")

    # collective bounce buffers (internal DRAM)
    zin_b = nc.dram_tensor("zin_b", [4, K * NB, 128], DT.float32)
    zout_b = nc.dram_tensor("zout_b", [NCORES, 4, K * NB, 128], DT.float32,
                            addr_space="Shared")

    from contextlib import ExitStack
    with tile.TileContext(nc) as tc, ExitStack() as stack:
        wp = stack.enter_context(tc.tile_pool(name="weights", bufs=1))
        persist = stack.enter_context(tc.tile_pool(name="persist", bufs=1))
        y1p_pool = stack.enter_context(tc.tile_pool(name="y1p", bufs=2))
        xcp = stack.enter_context(tc.tile_pool(name="xc", bufs=3))
        hp = stack.enter_context(tc.tile_pool(name="h", bufs=6))
        y2p = stack.enter_context(tc.tile_pool(name="y2", bufs=3))
        psp = stack.enter_context(tc.tile_pool(name="psum", bufs=6, space="PSUM"))
        sp = stack.enter_context(tc.tile_pool(name="small", bufs=2))

        def wtile(dram, shape, dt):
            t = wp.tile(shape, dt, tag=dram.name)
            nc.sync.dma_start(t[:], dram[:])
            return t

        w1T = wtile(w1T_d, [128, DIM], DT.bfloat16)
        r1T = wtile(r1T_d, [128, R, 4, HALF], DT.bfloat16)
        w2T = wtile(w2T_d, [128, R, 9, 2, HALF], DT.bfloat16)
        r3T = wtile(r3T_d, [128, R, 2, DIM], DT.bfloat16)
        encb = wtile(encb_d, [128, 4], DT.float32)
        b1 = wtile(b1_d, [128, R, 2], DT.float32)
        b2 = wtile(b2_d, [128, R, 2], DT.float32)
        b3 = wtile(b3_d, [128, R, 4], DT.float32)
        gihT = wtile(gihT_d, [128, 4, 3 * HID], DT.float32)
        ghhT = wtile(ghhT_d, [128, 2, 3 * HID], DT.float32)
        gbih = wtile(gbih_d, [NB, 3 * HID], DT.float32)
        gbhh = wtile(gbhh_d, [NB, 3 * HID], DT.float32)
        wkT = wtile(wkT_d, [128, K, 2, DIM], DT.float32)
        wkb = wtile(wkb_d, [128, K, 4], DT.float32)
        mask = wtile(mask_d, [NB, K * B], DT.float32)
        ident = wtile(ident_d, [NB, NB], DT.float32)
        iota64 = wtile(iota_d, [NB, B], DT.float32)

        zbuf = persist.tile([128, 4, NIMG], DT.float32, tag="zbuf")
        y1pads = [y1p_pool.tile([128, 2, 2, 18, 18], DT.bfloat16, tag="y1pad",
                                name=f"y1pad{i}") for i in range(2)]
        for y in y1pads:
            nc.vector.memset(y[:], 0.0)

        # ---- conv encoder over image pairs ----
        for p in range(NPAIR):
            xc = xcp.tile([128, 512], DT.bfloat16, tag="xc")
            nc.sync.dma_start(xc[:], xcol_d[p])

            h = hp.tile([128, 4, 512], DT.bfloat16, tag="h")
            for m in range(4):
                ps = psp.tile([128, 512], DT.float32, tag="ps")
                nc.tensor.matmul(ps[:], w1T[:, m * 128:(m + 1) * 128], xc[:],
                                 start=True, stop=True)
                nc.scalar.activation(h[:, m], ps[:], ACTF.Relu,
                                     bias=encb[:, m:m + 1])

            for r in range(R):
                y1p = y1pads[p % 2]
                for m in range(2):
                    ps = psp.tile([128, 512], DT.float32, tag="ps")
                    for c in range(4):
                        nc.tensor.matmul(ps[:], r1T[:, r, c, m * 128:(m + 1) * 128],
                                         h[:, c], start=(c == 0), stop=(c == 3))
                    nc.scalar.activation(y1p[:, m, :, 1:17, 1:17],
                                         ps[:].rearrange("p (i r c) -> p i r c",
                                                         i=2, r=16),
                                         ACTF.Relu, bias=b1[:, r, m:m + 1])
                y2 = y2p.tile([128, 2, 512], DT.bfloat16, tag="y2")
                for m in range(2):
                    ps = psp.tile([128, 512], DT.float32, tag="ps")
                    n_mm = 18
                    i_mm = 0
                    for tap in range(9):
                        ky, kx = divmod(tap, 3)
                        for c in range(2):
                            rhs = y1p[:, c, :, ky:ky + 16, kx:kx + 16]
                            nc.tensor.matmul(
                                ps[:].rearrange("p (i r c) -> p i r c", i=2, r=16),
                                w2T[:, r, tap, c, m * 128:(m + 1) * 128],
                                rhs, start=(i_mm == 0), stop=(i_mm == n_mm - 1))
                            i_mm += 1
                    nc.scalar.activation(y2[:, m], ps[:], ACTF.Relu,
                                         bias=b2[:, r, m:m + 1])
                hn = hp.tile([128, 4, 512], DT.bfloat16, tag="h")
                for m in range(4):
                    ps = psp.tile([128, 512], DT.float32, tag="ps")
                    for c in range(2):
                        nc.tensor.matmul(ps[:], r3T[:, r, c, m * 128:(m + 1) * 128],
                                         y2[:, c], start=(c == 0), stop=(c == 1))
                    nc.vector.tensor_add(hn[:, m], ps[:], h[:, m])
                    nc.scalar.activation(hn[:, m], hn[:, m], ACTF.Relu,
                                         bias=b3[:, r, m:m + 1])
                h = hn

            # avg pool both images -> zbuf cols [t*8+2j, +1]
            t_idx, j_idx = divmod(p, 4)
            col = t_idx * 8 + 2 * j_idx
            zt = sp.tile([128, 4, 2], DT.float32, tag="zt")
            nc.vector.tensor_reduce(zt[:], h[:].rearrange("p c (i x) -> p c i x",
                                                          i=2),
                                    mybir.AxisListType.X, ALU.add)
            nc.vector.tensor_scalar_mul(zbuf[:, :, col:col + 2], zt[:],
                                        1.0 / NPIX)

        # ---- ztk AllGather (overlaps GRU on the collective engine) ----
        if stage == 'conv':
            dbg = persist.tile([NB, 2 * K], DT.float32, tag="dbg")
            nc.vector.tensor_copy(dbg[:], zbuf[:NB, 0, :2 * K])
            nc.sync.dma_start(out_d[:], dbg[:])
        do_cc = stage in ('cc', 'full', 'sc', 'lse')
        for c in range(4 if do_cc else 0):
            nc.sync.dma_start(zin_b[c].rearrange("n p -> p n"),
                              zbuf[:, c, TCTX * 8:])
        if do_cc:
            nc.gpsimd.collective_compute(
                "AllGather", ALU.bypass,
                replica_groups=[list(range(NCORES))],
                ins=[zin_b.ap().opt()], outs=[zout_b.ap().opt()])
        ztk = persist.tile([128, NCORES, 4, K * NB], DT.float32, tag="ztk")
        for core in range(NCORES if do_cc else 0):
            nc.sync.dma_start(
                ztk[:, core],
                zout_b[core].rearrange("c n p -> p c n"))

        # ---- GRU over t = 0..3 (f32) ----
        do_gru = stage != 'conv'
        gp = sp  # reuse small pool
        hT = persist.tile([128, 2, NB], DT.float32, tag="hT")
        h_cur = None
        for t in range(TCTX if do_gru else 0):
            gates = []
            for g in range(3):
                gi = psp.tile([NB, HID], DT.float32, tag="ps")
                for c in range(4):
                    nc.tensor.matmul(gi[:], zbuf[:, c, t * 8:(t + 1) * 8],
                                     gihT[:, c, g * HID:(g + 1) * HID],
                                     start=(c == 0), stop=(c == 3))
                gates.append(gi)
            ghs = []
            if t > 0:
                for g in range(3):
                    gh = psp.tile([NB, HID], DT.float32, tag="ps")
                    for c in range(2):
                        nc.tensor.matmul(gh[:], hT[:, c, :],
                                         ghhT[:, c, g * HID:(g + 1) * HID],
                                         start=(c == 0), stop=(c == 1))
                    ghs.append(gh)

            def gsum(g, dst):
                # dst = gi_g + b_ih_g (+ gh_g)  ... b_hh added separately below
                nc.vector.tensor_add(dst[:], gates[g][:],
                                     gbih[:, g * HID:(g + 1) * HID])
                if t > 0:
                    nc.vector.tensor_add(dst[:], dst[:], ghs[g][:])

            rg = gp.tile([NB, HID], DT.float32, tag="rg")
            zg = gp.tile([NB, HID], DT.float32, tag="zg")
            ng = gp.tile([NB, HID], DT.float32, tag="ng")
            tmp = gp.tile([NB, HID], DT.float32, tag="gtmp")
            # r = sigmoid(ir + hr + bih_r + bhh_r)
            gsum(0, rg)
            nc.vector.tensor_add(rg[:], rg[:], gbhh[:, 0:HID])
            nc.scalar.activation(rg[:], rg[:], ACTF.Sigmoid)
            # z = sigmoid(iz + hz + bih_z + bhh_z)
            gsum(1, zg)
            nc.vector.tensor_add(zg[:], zg[:], gbhh[:, HID:2 * HID])
            nc.scalar.activation(zg[:], zg[:], ACTF.Sigmoid)
            # n = tanh(inn + bih_n + r * (hn + bhh_n))
            if t > 0:
                nc.vector.tensor_add(tmp[:], ghs[2][:], gbhh[:, 2 * HID:])
            else:
                nc.vector.tensor_copy(tmp[:], gbhh[:, 2 * HID:])
            nc.vector.tensor_mul(tmp[:], tmp[:], rg[:])
            nc.vector.tensor_add(ng[:], gates[2][:], gbih[:, 2 * HID:])
            nc.vector.tensor_add(ng[:], ng[:], tmp[:])
            nc.scalar.activation(ng[:], ng[:], ACTF.Tanh)
            # h = (1-z)*n + z*h_prev
            h_new = gp.tile([NB, HID], DT.float32, tag=f"hstep{t}")
            if t == 0:
                nc.vector.tensor_mul(tmp[:], zg[:], ng[:])
                nc.vector.tensor_sub(h_new[:], ng[:], tmp[:])
            else:
                nc.vector.tensor_sub(tmp[:], h_cur[:], ng[:])
                nc.vector.tensor_mul(tmp[:], zg[:], tmp[:])
                nc.vector.tensor_add(h_new[:], ng[:], tmp[:])
            h_cur = h_new
            # hT for next step / preds
            for c in range(2):
                pt = psp.tile([128, NB], DT.float32, tag="ps")
                nc.tensor.transpose(pt[:], h_new[:, c * 128:(c + 1) * 128],
                                    ident[:])
                nc.vector.tensor_copy(hT[:, c, :], pt[:])

        # ---- preds[d, k*8+i] = wk[k] @ ct ----
        preds = persist.tile([128, 4, K * NB], DT.float32, tag="preds")
        for k in range(K if do_gru else 0):
            for m in range(4):
                pp = psp.tile([128, NB], DT.float32, tag="ps")
                for c in range(2):
                    nc.tensor.matmul(pp[:], wkT[:, k, c, m * 128:(m + 1) * 128],
                                     hT[:, c, :], start=(c == 0), stop=(c == 1))
                nc.scalar.activation(preds[:, m, k * NB:(k + 1) * NB], pp[:],
                                     ACTF.Identity, bias=wkb[:, k, m:m + 1])

        if stage in ('gru', 'cc'):
            dbg2 = persist.tile([NB, 2 * K], DT.float32, tag="dbg2")
            nc.vector.tensor_copy(dbg2[:], preds[:NB, 0, :2 * K])
            if do_cc:
                nc.vector.tensor_add(dbg2[:], dbg2[:], ztk[:NB, 0, 0, :2 * K])
            nc.sync.dma_start(out_d[:], dbg2[:])
        # ---- scores + double-exp log-softmax ----
        do_loss = stage in ('full', 'sc', 'lse')
        do_lse = stage in ('full', 'lse')
        do_amax = stage == 'full'
        sc = psp.tile([NB, K * B], DT.float32, tag="ps")
        for k in range(K if do_loss else 0):
            for c in range(4):
                nc.tensor.matmul(sc[:, k * B:(k + 1) * B],
                                 preds[:, c, k * NB:(k + 1) * NB],
                                 ztk[:, :, c, k * NB:(k + 1) * NB],
                                 start=(c == 0), stop=(c == 3))
        y = persist.tile([NB, K * B], DT.float32, tag="y")
        if do_loss:
            nc.scalar.activation(y[:], sc[:], ACTF.Exp)

        out_sb = persist.tile([NB, 2 * K], DT.float32, tag="out_sb")
        if do_loss:
            nc.vector.memset(out_sb[:], 0.0)
            if stage == 'sc':
                nc.vector.tensor_copy(out_sb[:], y[:, :2 * K])
        for k in range(K if do_lse else 0):
            yk = y[:, k * B:(k + 1) * B]
            mk = sp.tile([NB, 1], DT.float32, tag="mk")
            nmk = sp.tile([NB, 1], DT.float32, tag="nmk")
            ek = sp.tile([NB, B], DT.float32, tag="ek")
            sek = sp.tile([NB, 1], DT.float32, tag="sek")
            lgk = sp.tile([NB, 1], DT.float32, tag="lgk")
            dgk = sp.tile([NB, 1], DT.float32, tag="dgk")
            nc.vector.tensor_reduce(mk[:], yk, mybir.AxisListType.X, ALU.max)
            nc.vector.tensor_scalar_mul(nmk[:], mk[:], -1.0)
            nc.scalar.activation(ek[:], yk, ACTF.Exp, bias=nmk[:])
            nc.vector.tensor_reduce(sek[:], ek[:], mybir.AxisListType.X, ALU.add)
            nc.scalar.activation(lgk[:], sek[:], ACTF.Ln)
            nc.vector.tensor_add(lgk[:], lgk[:], mk[:])   # lse
            nc.vector.tensor_mul(ek[:], yk, mask[:, k * B:(k + 1) * B])
            nc.vector.tensor_reduce(dgk[:], ek[:], mybir.AxisListType.X, ALU.add)
            nc.vector.tensor_sub(out_sb[:, k:k + 1], dgk[:], lgk[:])
            if do_amax:
                # argmax: onehot(y == max) dotted with iota
                eq = sp.tile([NB, B], DT.float32, tag="eq")
                nc.vector.tensor_scalar(eq[:], yk, mk[:], 0.0,
                                        ALU.subtract, ALU.is_equal)
                nc.vector.tensor_mul(eq[:], eq[:], iota64[:])
                nc.vector.tensor_reduce(out_sb[:, K + k:K + k + 1], eq[:],
                                        mybir.AxisListType.X, ALU.add)

        if do_loss:
            nc.sync.dma_start(out_d[:], out_sb[:])

    nc.compile()
    return nc


def host_prep(inputs):
    """Host-side prep: im2col for conv1, weight layout transforms, bf16 casts."""
    x = np.asarray(inputs['x'], F32)
    xp = np.pad(x, ((0, 0), (0, 0), (0, 0), (2, 2), (2, 2)))
    s = xp.strides
    xs = np.lib.stride_tricks.as_strided(
        xp, shape=(B, T, C, 5, 5, 16, 16),
        strides=(s[0], s[1], s[2], s[3], s[4], 2 * s[3], 2 * s[4]))
    x_col = np.ascontiguousarray(xs).reshape(B, T, 75, NPIX).astype(BF16)

    xcols = []
    for core in range(NCORES):
        xc = x_col[core * NB:(core + 1) * NB]
        arr = np.zeros((NPAIR, 128, 2 * NPIX), BF16)
        for t in range(T):
            for j in range(NB // 2):
                p = t * 4 + j
                arr[p, :75, :NPIX] = xc[2 * j, t]
                arr[p, :75, NPIX:] = xc[2 * j + 1, t]
        xcols.append(arr)

    w = {}
    w1T = np.zeros((128, DIM), BF16)
    w1T[:75] = np.asarray(inputs['enc_w'], F32).reshape(DIM, 75).T.astype(BF16)
    w['w1T'] = w1T
    r1 = np.asarray(inputs['res_w1'], F32).reshape(R, HALF, DIM).transpose(0, 2, 1)
    w['r1T'] = np.ascontiguousarray(
        r1.reshape(R, 4, 128, HALF).transpose(2, 0, 1, 3)).astype(BF16)
    r2 = np.asarray(inputs['res_w2'], F32).transpose(0, 3, 4, 2, 1)
    w['w2T'] = np.ascontiguousarray(
        r2.reshape(R, 9, 2, 128, HALF).transpose(3, 0, 1, 2, 4)).astype(BF16)
    r3 = np.asarray(inputs['res_w3'], F32).reshape(R, DIM, HALF).transpose(0, 2, 1)
    w['r3T'] = np.ascontiguousarray(
        r3.reshape(R, 2, 128, DIM).transpose(2, 0, 1, 3)).astype(BF16)
    w['encb'] = np.ascontiguousarray(
        np.asarray(inputs['enc_b'], F32).reshape(4, 128).T)
    w['b1'] = np.ascontiguousarray(
        np.asarray(inputs['res_b1'], F32).reshape(R, 2, 128).transpose(2, 0, 1))
    w['b2'] = np.ascontiguousarray(
        np.asarray(inputs['res_b2'], F32).reshape(R, 2, 128).transpose(2, 0, 1))
    w['b3'] = np.ascontiguousarray(
        np.asarray(inputs['res_b3'], F32).reshape(R, 4, 128).transpose(2, 0, 1))
    w['gihT'] = np.ascontiguousarray(
        np.asarray(inputs['gru_w_ih'], F32).T.reshape(4, 128, 3 * HID)
        .transpose(1, 0, 2))
    w['ghhT'] = np.ascontiguousarray(
        np.asarray(inputs['gru_w_hh'], F32).T.reshape(2, 128, 3 * HID)
        .transpose(1, 0, 2))
    w['gbih'] = np.tile(np.asarray(inputs['gru_b_ih'], F32)[None, :], (NB, 1))
    w['gbhh'] = np.tile(np.asarray(inputs['gru_b_hh'], F32)[None, :], (NB, 1))
    wk = np.asarray(inputs['wk_w'], F32).transpose(0, 2, 1)
    w['wkT'] = np.ascontiguousarray(
        wk.reshape(K, 2, 128, DIM).transpose(2, 0, 1, 3))
    w['wkb'] = np.ascontiguousarray(
        np.asarray(inputs['wk_b'], F32).reshape(K, 4, 128).transpose(2, 0, 1))
    w['ident8'] = np.eye(NB, dtype=F32)
    w['iota64'] = np.tile(np.arange(B, dtype=F32)[None, :], (NB, 1))
    return xcols, w


_NC_CACHE = {}


def get_nc(stage='full'):
    if stage not in _NC_CACHE:
        _NC_CACHE[stage] = build_kernel(stage)
    return _NC_CACHE[stage]


def make_in_maps(inputs):
    xcols, w = host_prep(inputs)
    in_maps = []
    for core in range(NCORES):
        m = dict(w)
        m['xcol'] = xcols[core]
        msk = np.zeros((NB, K, B), F32)
        for i in range(NB):
            msk[i, :, core * NB + i] = 1.0
        m['mask'] = msk.reshape(NB, K * B)
        in_maps.append(m)
    return in_maps


def reduce_outputs(results):
    tot, correct = 0.0, 0
    for core in range(NCORES):
        o = np.asarray(results[core]['out'], F32)
        tot += float(o[:, :K].sum())
        idx = o[:, K:]
        for i in range(NB):
            correct += int((idx[i] == core * NB + i).sum())
    loss = np.float32(-tot / (B * K))
    acc = np.float32(correct / (B * K))
    return loss, acc


def _install_ntff_hook():
    """Provide antenv.axon_hooks (missing in this image) so trace=True works.

    Mirrors trn_agent_boot._ntff_profile_via_ctypes: drives NRT profiling via
    the injected libaxon_pjrt.so C ABI.
    """
    try:
        from antenv.axon_hooks import get_axon_ntff_profile_hook  # noqa: F401
        return
    except ImportError:
        pass
    import ctypes
    import types
    import contextlib

    so_path = "/opt/axon/libaxon_pjrt.so"
    if not os.path.exists(so_path):
        return
    lib = ctypes.CDLL(so_path)
    if not hasattr(lib, "axon_start_nrt_profile"):
        return
    lib.axon_start_nrt_profile.argtypes = [ctypes.POINTER(ctypes.c_int64),
                                           ctypes.c_size_t]
    lib.axon_start_nrt_profile.restype = ctypes.c_int64
    lib.axon_stop_nrt_profile.argtypes = [ctypes.c_char_p]
    lib.axon_stop_nrt_profile.restype = ctypes.c_int64

    @contextlib.contextmanager
    def _hook(output_dir, device_ids):
        import jax
        jax.devices()
        if device_ids:
            ids = (ctypes.c_int64 * len(device_ids))(*device_ids)
            rc = lib.axon_start_nrt_profile(ids, len(device_ids))
        else:
            rc = lib.axon_start_nrt_profile(None, 0)
        if rc != 0:
            raise RuntimeError(f"axon_start_nrt_profile rc={rc}")
        try:
            yield
        finally:
            n = lib.axon_stop_nrt_profile(str(output_dir).encode())
            print(f"ntff profile: {n} file(s) written to {output_dir}")

    mod = types.ModuleType("antenv.axon_hooks")
    mod.get_axon_ntff_profile_hook = lambda: _hook
    mod.set_axon_ntff_profile_hook = lambda h: None
    import antenv
    antenv.axon_hooks = mod
    sys.modules["antenv.axon_hooks"] = mod


def run(inputs, trace=False, stage='full', **kw):
    if trace:
        _install_ntff_hook()
    nc = get_nc(stage)
    in_maps = make_in_maps(inputs)
    res = run_bass_kernel_spmd(nc, in_maps, core_ids=list(range(NCORES)),
                               trace=trace, **kw)
    return res


def kernel(**inputs):
    res = run(inputs, trace=False)
    return reduce_outputs(res.results)


if __name__ == '__main__':
    import reference as Rf
    inputs = {k: np.asarray(v) for k, v in Rf.setup_inputs().items()}
    loss, acc = kernel(**inputs)
    print('kernel loss/acc:', loss, acc)


# revision 17
# speedup vs baseline: 1.2060x; 1.2060x over previous
"""CPC model (conv encoder + GRU + InfoNCE loss) on 8 TRN2 NeuronCores.

Strategy:
 - Data-parallel over batch: each core owns 8 of 64 sequences (72 images).
 - Conv encoder runs per image-pair as bf16 matmuls (f32 PSUM accum):
     conv1 5x5s2 via host im2col (K=75 padded to 128), resblock 3x3 via
     9-tap shifted matmuls over a zero-padded [18,18] activation.
 - Timestep rows processed in order [4..8, 0..3]: the ztk rows finish early
   so the AllGather overlaps the remaining conv; GRU step t is emitted right
   after row t so it hides under the next row's conv.
 - Global avg pool -> z in f32; GRU, W_k preds, scores and the double-exp
   log-softmax all in f32.  Loss math batched on a [40(k*8+i), 64] layout.
 - Each core emits [40, 2] = (diag log-softmax value, argmax index); host
   does the final mean / accuracy.
"""
import os
import sys

import numpy as np
import ml_dtypes

for _p in ("/opt/trn_rl_repo", "/root/.axon_site/_ro/trn_rl_repo"):
    if os.path.isdir(_p) and _p not in sys.path:
        sys.path.insert(0, _p)

import concourse.bacc as bacc  # noqa: E402
import concourse.bass as bass  # noqa: E402
import concourse.mybir as mybir  # noqa: E402
import concourse.tile as tile  # noqa: E402
from concourse.bass_utils import run_bass_kernel_spmd  # noqa: E402

F32 = np.float32
BF16 = ml_dtypes.bfloat16
DT = mybir.dt

B, T, C = 64, 9, 3
DIM, HALF, HID, R, K = 512, 256, 256, 2, 5
TCTX = 4
NCORES = 8
NB = B // NCORES           # 8
NIMG = NB * T              # 72
NPIX = 256                 # 16*16
NPAIR = NIMG // 2          # 36
ROWS = [4, 5, 6, 7, 8, 0, 1, 2, 3]
ALU = mybir.AluOpType
ACTF = mybir.ActivationFunctionType


def build_kernel():
    nc = bacc.Bacc("TRN2", target_bir_lowering=False, debug=False,
                   num_devices=NCORES)

    def din(name, shape, dt):
        return nc.dram_tensor(name, shape, dt, kind="ExternalInput")

    xcol_d = din("xcol", [NPAIR, 128, 512], DT.bfloat16)
    w1T_d = din("w1T", [128, DIM], DT.bfloat16)
    r1T_d = din("r1T", [128, R, 4, HALF], DT.bfloat16)
    w2T_d = din("w2T", [128, R, 9, 2, HALF], DT.bfloat16)
    r3T_d = din("r3T", [128, R, 2, DIM], DT.bfloat16)
    encb_d = din("encb", [128, 4], DT.float32)
    b1_d = din("b1", [128, R, 2], DT.float32)
    b2_d = din("b2", [128, R, 2], DT.float32)
    b3_d = din("b3", [128, R, 4], DT.float32)
    gihT_d = din("gihT", [128, 4, 3 * HID], DT.float32)
    ghhT_d = din("ghhT", [128, 2, 3 * HID], DT.float32)
    gbih_d = din("gbih", [NB, 3 * HID], DT.float32)   # host-broadcast over batch
    gbhh_d = din("gbhh", [NB, 3 * HID], DT.float32)
    gbc_d = din("gbc", [NB, 2 * HID], DT.float32)     # (b_ih + b_hh)[: 512]
    wkT_d = din("wkT", [128, K, 2, DIM], DT.float32)
    wkb_d = din("wkb", [128, K, 4], DT.float32)
    mask_d = din("mask", [NB, K * B], DT.float32)     # one-hot diag per (i,k)
    iota_d = din("iota64", [NB, B], DT.float32)       # 0..63 per row
    ident_d = din("ident8", [NB, NB], DT.float32)

    out_d = nc.dram_tensor("out", [NB, 2 * K], DT.float32, kind="ExternalOutput")

    # collective bounce buffers (internal DRAM, partition-major for cheap DMA)
    zin_b = nc.dram_tensor("zin_b", [128, 4, K * NB], DT.float32)
    zout_b = nc.dram_tensor("zout_b", [NCORES, 128, 4, K * NB], DT.float32,
                            addr_space="Shared")

    from contextlib import ExitStack
    with tile.TileContext(nc) as tc, ExitStack() as stack:
        wp = stack.enter_context(tc.tile_pool(name="weights", bufs=1))
        persist = stack.enter_context(tc.tile_pool(name="persist", bufs=1))
        y1p_pool = stack.enter_context(tc.tile_pool(name="y1p", bufs=2))
        xcp = stack.enter_context(tc.tile_pool(name="xc", bufs=4))
        hp = stack.enter_context(tc.tile_pool(name="h", bufs=6))
        y2p = stack.enter_context(tc.tile_pool(name="y2", bufs=3))
        psp = stack.enter_context(tc.tile_pool(name="psum", bufs=8, space="PSUM"))
        sp = stack.enter_context(tc.tile_pool(name="small", bufs=2))

        def wtile(dram, shape, dt, split_dim1=False):
            t = wp.tile(shape, dt, tag=dram.name, name=f"w_{dram.name}")
            if split_dim1:
                for i in range(shape[1]):
                    nc.scalar.dma_start(t[:, i], dram[:, i])
            else:
                nc.scalar.dma_start(t[:], dram[:])
            return t

        # conv weights first (needed by pair 0); w2T split per-resblock
        w1T = wtile(w1T_d, [128, DIM], DT.bfloat16)
        encb = wtile(encb_d, [128, 4], DT.float32)
        r1T = wtile(r1T_d, [128, R, 4, HALF], DT.bfloat16)
        b1 = wtile(b1_d, [128, R, 2], DT.float32)
        w2T = wtile(w2T_d, [128, R, 9, 2, HALF], DT.bfloat16, split_dim1=True)
        b2 = wtile(b2_d, [128, R, 2], DT.float32)
        r3T = wtile(r3T_d, [128, R, 2, DIM], DT.bfloat16)
        b3 = wtile(b3_d, [128, R, 4], DT.float32)
        # gru / loss weights (needed only after row t=0)
        gihT = wtile(gihT_d, [128, 4, 3 * HID], DT.float32)
        ghhT = wtile(ghhT_d, [128, 2, 3 * HID], DT.float32)
        gbih = wtile(gbih_d, [NB, 3 * HID], DT.float32)
        gbhh = wtile(gbhh_d, [NB, 3 * HID], DT.float32)
        gbc = wtile(gbc_d, [NB, 2 * HID], DT.float32)
        wkT = wtile(wkT_d, [128, K, 2, DIM], DT.float32)
        wkb = wtile(wkb_d, [128, K, 4], DT.float32)
        mask = wtile(mask_d, [NB, K * B], DT.float32)
        iota64 = wtile(iota_d, [NB, B], DT.float32)
        ident = wtile(ident_d, [NB, NB], DT.float32)

        zbuf = persist.tile([128, 4, NIMG], DT.float32, tag="zbuf")
        hT = persist.tile([128, 2, NB], DT.float32, tag="hT")
        y40 = persist.tile([NB, K * B], DT.float32, tag="y40")
        out_sb = persist.tile([NB, 2 * K], DT.float32, tag="out_sb")
        y1pads = [y1p_pool.tile([128, 2, 2, 18, 18], DT.bfloat16, tag="y1pad",
                                name=f"y1pad{i}") for i in range(2)]
        for ypad in y1pads:
            nc.vector.memset(ypad[:], 0.0)

        # ---------------- conv encoder: one image pair ----------------
        def emit_pair(p):
            xc = xcp.tile([128, 512], DT.bfloat16, tag="xc", name=f"xc{p}")
            nc.sync.dma_start(xc[:], xcol_d[p])

            h = hp.tile([128, 4, 512], DT.bfloat16, tag="h", name=f"h{p}")
            for m in range(4):
                ps = psp.tile([128, 512], DT.float32, tag="ps", name=f"c1ps{p}{m}")
                nc.tensor.matmul(ps[:], w1T[:, m * 128:(m + 1) * 128], xc[:],
                                 start=True, stop=True)
                nc.scalar.activation(h[:, m], ps[:], ACTF.Relu,
                                     bias=encb[:, m:m + 1])

            for r in range(R):
                y1p = y1pads[p % 2]
                for m in range(2):
                    ps = psp.tile([128, 512], DT.float32, tag="ps",
                                  name=f"a_ps{p}{r}{m}")
                    for c in range(4):
                        nc.tensor.matmul(ps[:], r1T[:, r, c, m * 128:(m + 1) * 128],
                                         h[:, c], start=(c == 0), stop=(c == 3))
                    nc.scalar.activation(y1p[:, m, :, 1:17, 1:17],
                                         ps[:].rearrange("p (i r c) -> p i r c",
                                                         i=2, r=16),
                                         ACTF.Relu, bias=b1[:, r, m:m + 1])
                y2 = y2p.tile([128, 2, 512], DT.bfloat16, tag="y2", name=f"y2_{p}{r}")
                for m in range(2):
                    ps = psp.tile([128, 512], DT.float32, tag="ps",
                                  name=f"b_ps{p}{r}{m}")
                    i_mm = 0
                    for c in range(2):
                        for tap in range(9):
                            ky, kx = divmod(tap, 3)
                            rhs = y1p[:, c, :, ky:ky + 16, kx:kx + 16]
                            nc.tensor.matmul(
                                ps[:].rearrange("p (i r c) -> p i r c", i=2, r=16),
                                w2T[:, r, tap, c, m * 128:(m + 1) * 128],
                                rhs, start=(i_mm == 0), stop=(i_mm == 17))
                            i_mm += 1
                    nc.scalar.activation(y2[:, m], ps[:], ACTF.Relu,
                                         bias=b2[:, r, m:m + 1])
                hn = hp.tile([128, 4, 512], DT.bfloat16, tag="h", name=f"hn{p}{r}")
                for m in range(4):
                    ps = psp.tile([128, 512], DT.float32, tag="ps",
                                  name=f"c_ps{p}{r}{m}")
                    for c in range(2):
                        nc.tensor.matmul(ps[:], r3T[:, r, c, m * 128:(m + 1) * 128],
                                         y2[:, c], start=(c == 0), stop=(c == 1))
                    nc.vector.tensor_add(hn[:, m], ps[:], h[:, m])
                    nc.scalar.activation(hn[:, m], hn[:, m], ACTF.Relu,
                                         bias=b3[:, r, m:m + 1])
                h = hn

            t_idx, j_idx = divmod(p, 4)
            col = t_idx * 8 + 2 * j_idx
            zt = sp.tile([128, 4, 2], DT.float32, tag="zt", name=f"zt{p}")
            nc.vector.tensor_reduce(zt[:], h[:].rearrange("p c (i x) -> p c i x",
                                                          i=2),
                                    mybir.AxisListType.X, ALU.add)
            nc.vector.tensor_scalar_mul(zbuf[:, :, col:col + 2], zt[:],
                                        1.0 / NPIX)

        # ---------------- GRU step (emitted after row t) ----------------
        gru_state = {'h': None}

        def emit_gru_step(t):
            gi_rz = psp.tile([NB, 2 * HID], DT.float32, tag="ps", name=f"girz{t}")
            gi_n = psp.tile([NB, HID], DT.float32, tag="ps", name=f"gin{t}")
            for c in range(4):
                nc.tensor.matmul(gi_rz[:], zbuf[:, c, t * 8:(t + 1) * 8],
                                 gihT[:, c, :2 * HID],
                                 start=(c == 0), stop=(c == 3))
            for c in range(4):
                nc.tensor.matmul(gi_n[:], zbuf[:, c, t * 8:(t + 1) * 8],
                                 gihT[:, c, 2 * HID:],
                                 start=(c == 0), stop=(c == 3))
            gh_rz = gh_n = None
            if t > 0:
                gh_rz = psp.tile([NB, 2 * HID], DT.float32, tag="ps",
                                 name=f"ghrz{t}")
                gh_n = psp.tile([NB, HID], DT.float32, tag="ps", name=f"ghn{t}")
                for c in range(2):
                    nc.tensor.matmul(gh_rz[:], hT[:, c, :], ghhT[:, c, :2 * HID],
                                     start=(c == 0), stop=(c == 1))
                for c in range(2):
                    nc.tensor.matmul(gh_n[:], hT[:, c, :], ghhT[:, c, 2 * HID:],
                                     start=(c == 0), stop=(c == 1))

            rz = sp.tile([NB, 2 * HID], DT.float32, tag="rz", name=f"rz{t}")
            ng = sp.tile([NB, HID], DT.float32, tag="ng", name=f"ng{t}")
            tmp = sp.tile([NB, HID], DT.float32, tag="gtmp", name=f"gtmp{t}")
            # r,z = sigmoid(gi_rz + gh_rz + (b_ih + b_hh)[:512])
            nc.vector.tensor_add(rz[:], gi_rz[:], gbc[:])
            if t > 0:
                nc.vector.tensor_add(rz[:], rz[:], gh_rz[:])
            nc.scalar.activation(rz[:], rz[:], ACTF.Sigmoid)
            # n = tanh(gi_n + b_ih_n + r * (gh_n + b_hh_n))
            if t > 0:
                nc.vector.tensor_add(tmp[:], gh_n[:], gbhh[:, 2 * HID:])
            else:
                nc.vector.tensor_copy(tmp[:], gbhh[:, 2 * HID:])
            nc.vector.tensor_mul(tmp[:], tmp[:], rz[:, :HID])
            nc.vector.tensor_add(ng[:], gi_n[:], gbih[:, 2 * HID:])
            nc.vector.tensor_add(ng[:], ng[:], tmp[:])
            nc.scalar.activation(ng[:], ng[:], ACTF.Tanh)
            # h = (1-z)*n + z*h_prev
            h_new = sp.tile([NB, HID], DT.float32, tag=f"hstep{t}",
                            name=f"hnew{t}")
            if t == 0:
                nc.vector.tensor_mul(tmp[:], rz[:, HID:], ng[:])
                nc.vector.tensor_sub(h_new[:], ng[:], tmp[:])
            else:
                nc.vector.tensor_sub(tmp[:], gru_state['h'][:], ng[:])
                nc.vector.tensor_mul(tmp[:], rz[:, HID:], tmp[:])
                nc.vector.tensor_add(h_new[:], ng[:], tmp[:])
            gru_state['h'] = h_new

        def emit_transposes(t):
            h_new = gru_state['h']
            for c in range(2):
                pt = psp.tile([128, NB], DT.float32, tag="ps", name=f"pt{t}{c}")
                nc.tensor.transpose(pt[:], h_new[:, c * 128:(c + 1) * 128],
                                    ident[:])
                nc.vector.tensor_copy(hT[:, c, :], pt[:])

        # -------- emit: conv rows with GRU / collective interleaved --------
        pending = []
        ztk = persist.tile([128, NCORES, 4, K * NB], DT.float32, tag="ztk")
        for t in ROWS:
            for j in range(4):
                emit_pair(t * 4 + j)
                if j == 0 and pending:
                    for fn in pending:
                        fn()
                    pending = []
            if t == 8:
                # ztk rows complete -> AllGather (gpsimd queue, overlaps conv)
                nc.gpsimd.dma_start(zin_b.ap(), zbuf[:, :, TCTX * 8:])
                nc.gpsimd.collective_compute(
                    "AllGather", ALU.bypass,
                    replica_groups=[list(range(NCORES))],
                    ins=[zin_b.ap().opt()], outs=[zout_b.ap().opt()])
                for core in range(NCORES):
                    nc.gpsimd.dma_start(ztk[:, core], zout_b[core])
            if t <= 3:
                emit_gru_step(t)
                if t < 3:
                    pending.append(lambda t=t: emit_transposes(t))
                else:
                    emit_transposes(t)

        # ---------------- preds + scores + loss ----------------
        preds = persist.tile([128, 4, K * NB], DT.float32, tag="preds")
        for k in range(K):
            for m in range(4):
                pp = psp.tile([128, NB], DT.float32, tag="ps", name=f"pp{k}{m}")
                for c in range(2):
                    nc.tensor.matmul(pp[:], wkT[:, k, c, m * 128:(m + 1) * 128],
                                     hT[:, c, :], start=(c == 0), stop=(c == 1))
                nc.scalar.activation(preds[:, m, k * NB:(k + 1) * NB], pp[:],
                                     ACTF.Identity, bias=wkb[:, k, m:m + 1])

        for k in range(K):
            psk = psp.tile([NB, B], DT.float32, tag="ps", name=f"sck{k}")
            for c in range(4):
                nc.tensor.matmul(psk[:], preds[:, c, k * NB:(k + 1) * NB],
                                 ztk[:, :, c, k * NB:(k + 1) * NB],
                                 start=(c == 0), stop=(c == 3))
            nc.scalar.activation(y40[:, k * B:(k + 1) * B], psk[:], ACTF.Exp)

        for k in range(K):
            yk = y40[:, k * B:(k + 1) * B]
            mk = sp.tile([NB, 1], DT.float32, tag="mk", name=f"mk{k}")
            nmk = sp.tile([NB, 1], DT.float32, tag="nmk", name=f"nmk{k}")
            ek = sp.tile([NB, B], DT.float32, tag="ek", name=f"ek{k}")
            sek = sp.tile([NB, 1], DT.float32, tag="sek", name=f"sek{k}")
            lgk = sp.tile([NB, 1], DT.float32, tag="lgk", name=f"lgk{k}")
            dgk = sp.tile([NB, 1], DT.float32, tag="dgk", name=f"dgk{k}")
            eq = sp.tile([NB, B], DT.float32, tag="eq", name=f"eq{k}")
            nc.vector.tensor_reduce(mk[:], yk, mybir.AxisListType.X, ALU.max)
            nc.vector.tensor_scalar_mul(nmk[:], mk[:], -1.0)
            nc.scalar.activation(ek[:], yk, ACTF.Exp, bias=nmk[:])
            nc.vector.tensor_reduce(sek[:], ek[:], mybir.AxisListType.X, ALU.add)
            nc.scalar.activation(lgk[:], sek[:], ACTF.Ln)
            nc.vector.tensor_add(lgk[:], lgk[:], mk[:])     # lse
            nc.vector.tensor_mul(ek[:], yk, mask[:, k * B:(k + 1) * B])
            nc.vector.tensor_reduce(dgk[:], ek[:], mybir.AxisListType.X, ALU.add)
            nc.vector.tensor_sub(out_sb[:, k:k + 1], dgk[:], lgk[:])
            # argmax: onehot(y == max) dotted with iota
            nc.vector.tensor_scalar(eq[:], yk, mk[:], 0.0,
                                    ALU.subtract, ALU.is_equal)
            nc.vector.tensor_mul(eq[:], eq[:], iota64[:])
            nc.vector.tensor_reduce(out_sb[:, K + k:K + k + 1], eq[:],
                                    mybir.AxisListType.X, ALU.add)

        nc.sync.dma_start(out_d[:], out_sb[:])

    nc.compile()
    return nc


def host_prep(inputs):
    """Host-side prep: im2col for conv1, weight layout transforms, bf16 casts."""
    x = np.asarray(inputs['x'], F32)
    xp = np.pad(x, ((0, 0), (0, 0), (0, 0), (2, 2), (2, 2)))
    s = xp.strides
    xs = np.lib.stride_tricks.as_strided(
        xp, shape=(B, T, C, 5, 5, 16, 16),
        strides=(s[0], s[1], s[2], s[3], s[4], 2 * s[3], 2 * s[4]))
    x_col = np.ascontiguousarray(xs).reshape(B, T, 75, NPIX).astype(BF16)

    xcols = []
    for core in range(NCORES):
        xc = x_col[core * NB:(core + 1) * NB]
        arr = np.zeros((NPAIR, 128, 2 * NPIX), BF16)
        for t in range(T):
            for j in range(NB // 2):
                p = t * 4 + j
                arr[p, :75, :NPIX] = xc[2 * j, t]
                arr[p, :75, NPIX:] = xc[2 * j + 1, t]
        xcols.append(arr)

    w = {}
    w1T = np.zeros((128, DIM), BF16)
    w1T[:75] = np.asarray(inputs['enc_w'], F32).reshape(DIM, 75).T.astype(BF16)
    w['w1T'] = w1T
    r1 = np.asarray(inputs['res_w1'], F32).reshape(R, HALF, DIM).transpose(0, 2, 1)
    w['r1T'] = np.ascontiguousarray(
        r1.reshape(R, 4, 128, HALF).transpose(2, 0, 1, 3)).astype(BF16)
    r2 = np.asarray(inputs['res_w2'], F32).transpose(0, 3, 4, 2, 1)
    w['w2T'] = np.ascontiguousarray(
        r2.reshape(R, 9, 2, 128, HALF).transpose(3, 0, 1, 2, 4)).astype(BF16)
    r3 = np.asarray(inputs['res_w3'], F32).reshape(R, DIM, HALF).transpose(0, 2, 1)
    w['r3T'] = np.ascontiguousarray(
        r3.reshape(R, 2, 128, DIM).transpose(2, 0, 1, 3)).astype(BF16)
    w['encb'] = np.ascontiguousarray(
        np.asarray(inputs['enc_b'], F32).reshape(4, 128).T)
    w['b1'] = np.ascontiguousarray(
        np.asarray(inputs['res_b1'], F32).reshape(R, 2, 128).transpose(2, 0, 1))
    w['b2'] = np.ascontiguousarray(
        np.asarray(inputs['res_b2'], F32).reshape(R, 2, 128).transpose(2, 0, 1))
    w['b3'] = np.ascontiguousarray(
        np.asarray(inputs['res_b3'], F32).reshape(R, 4, 128).transpose(2, 0, 1))
    w['gihT'] = np.ascontiguousarray(
        np.asarray(inputs['gru_w_ih'], F32).T.reshape(4, 128, 3 * HID)
        .transpose(1, 0, 2))
    w['ghhT'] = np.ascontiguousarray(
        np.asarray(inputs['gru_w_hh'], F32).T.reshape(2, 128, 3 * HID)
        .transpose(1, 0, 2))
    bih = np.asarray(inputs['gru_b_ih'], F32)
    bhh = np.asarray(inputs['gru_b_hh'], F32)
    w['gbih'] = np.tile(bih[None, :], (NB, 1))
    w['gbhh'] = np.tile(bhh[None, :], (NB, 1))
    w['gbc'] = np.tile((bih + bhh)[None, :2 * HID], (NB, 1))
    wk = np.asarray(inputs['wk_w'], F32).transpose(0, 2, 1)
    w['wkT'] = np.ascontiguousarray(
        wk.reshape(K, 2, 128, DIM).transpose(2, 0, 1, 3))
    w['wkb'] = np.ascontiguousarray(
        np.asarray(inputs['wk_b'], F32).reshape(K, 4, 128).transpose(2, 0, 1))
    w['ident8'] = np.eye(NB, dtype=F32)
    w['iota64'] = np.tile(np.arange(B, dtype=F32)[None, :], (NB, 1))
    return xcols, w


_NC_CACHE = {}


def get_nc():
    if 'nc' not in _NC_CACHE:
        _NC_CACHE['nc'] = build_kernel()
    return _NC_CACHE['nc']


def make_in_maps(inputs):
    xcols, w = host_prep(inputs)
    in_maps = []
    for core in range(NCORES):
        m = dict(w)
        m['xcol'] = xcols[core]
        msk = np.zeros((NB, K, B), F32)
        for i in range(NB):
            msk[i, :, core * NB + i] = 1.0
        m['mask'] = msk.reshape(NB, K * B)
        in_maps.append(m)
    return in_maps


def reduce_outputs(results):
    tot, correct = 0.0, 0
    for core in range(NCORES):
        o = np.asarray(results[core]['out'], F32)   # [8, 10]
        tot += float(o[:, :K].sum())
        for i in range(NB):
            correct += int((o[i, K:] == core * NB + i).sum())
    loss = np.float32(-tot / (B * K))
    acc = np.float32(correct / (B * K))
    return loss, acc


def _install_ntff_hook():
    """Provide antenv.axon_hooks (missing in this image) so trace=True works."""
    try:
        from antenv.axon_hooks import get_axon_ntff_profile_hook  # noqa: F401
        return
    except ImportError:
        pass
    import ctypes
    import types
    import contextlib

    so_path = "/opt/axon/libaxon_pjrt.so"
    if not os.path.exists(so_path):
        return
    lib = ctypes.CDLL(so_path)
    if not hasattr(lib, "axon_start_nrt_profile"):
        return
    lib.axon_start_nrt_profile.argtypes = [ctypes.POINTER(ctypes.c_int64),
                                           ctypes.c_size_t]
    lib.axon_start_nrt_profile.restype = ctypes.c_int64
    lib.axon_stop_nrt_profile.argtypes = [ctypes.c_char_p]
    lib.axon_stop_nrt_profile.restype = ctypes.c_int64

    @contextlib.contextmanager
    def _hook(output_dir, device_ids):
        import jax
        jax.devices()
        if device_ids:
            ids = (ctypes.c_int64 * len(device_ids))(*device_ids)
            rc = lib.axon_start_nrt_profile(ids, len(device_ids))
        else:
            rc = lib.axon_start_nrt_profile(None, 0)
        if rc != 0:
            raise RuntimeError(f"axon_start_nrt_profile rc={rc}")
        try:
            yield
        finally:
            n = lib.axon_stop_nrt_profile(str(output_dir).encode())
            print(f"ntff profile: {n} file(s) written to {output_dir}")

    mod = types.ModuleType("antenv.axon_hooks")
    mod.get_axon_ntff_profile_hook = lambda: _hook
    mod.set_axon_ntff_profile_hook = lambda h: None
    import antenv
    antenv.axon_hooks = mod
    sys.modules["antenv.axon_hooks"] = mod


def run(inputs, trace=False, **kw):
    if trace:
        _install_ntff_hook()
    nc = get_nc()
    in_maps = make_in_maps(inputs)
    res = run_bass_kernel_spmd(nc, in_maps, core_ids=list(range(NCORES)),
                               trace=trace, **kw)
    return res


def kernel(**inputs):
    res = run(inputs, trace=False)
    return reduce_outputs(res.results)


if __name__ == '__main__':
    import reference as Rf
    inputs = {k: np.asarray(v) for k, v in Rf.setup_inputs().items()}
    loss, acc = kernel(**inputs)
    print('kernel loss/acc:', loss, acc)


# revision 18
# speedup vs baseline: 1.3706x; 1.1365x over previous
"""CPC model (conv encoder + GRU + InfoNCE loss) on 8 TRN2 NeuronCores.

Strategy:
 - Data-parallel over batch: each core owns 8 of 64 sequences (72 images).
 - Conv encoder runs per image-pair as bf16 matmuls (f32 PSUM accum):
     conv1 5x5s2 via host im2col (K=75 padded to 128), resblock 3x3 via
     9-tap shifted matmuls over a zero-padded [18,18] activation.
 - Timestep rows processed in order [4..8, 0..3]: the ztk rows finish early
   so the AllGather overlaps the remaining conv; GRU step t is emitted right
   after row t so it hides under the next row's conv.
 - Global avg pool -> z in f32; GRU, W_k preds, scores and the double-exp
   log-softmax all in f32.  Loss math batched on a [40(k*8+i), 64] layout.
 - Each core emits [40, 2] = (diag log-softmax value, argmax index); host
   does the final mean / accuracy.
"""
import os
import sys

import numpy as np
import ml_dtypes

for _p in ("/opt/trn_rl_repo", "/root/.axon_site/_ro/trn_rl_repo"):
    if os.path.isdir(_p) and _p not in sys.path:
        sys.path.insert(0, _p)

import concourse.bacc as bacc  # noqa: E402
import concourse.bass as bass  # noqa: E402
import concourse.mybir as mybir  # noqa: E402
import concourse.tile as tile  # noqa: E402
from concourse.bass_utils import run_bass_kernel_spmd  # noqa: E402

F32 = np.float32
BF16 = ml_dtypes.bfloat16
DT = mybir.dt

B, T, C = 64, 9, 3
DIM, HALF, HID, R, K = 512, 256, 256, 2, 5
TCTX = 4
NCORES = 8
NB = B // NCORES           # 8
NIMG = NB * T              # 72
NPIX = 256                 # 16*16
NPAIR = NIMG // 2          # 36
ROWS = [4, 5, 6, 7, 8, 0, 1, 2, 3]
ALU = mybir.AluOpType
ACTF = mybir.ActivationFunctionType


def build_kernel():
    nc = bacc.Bacc("TRN2", target_bir_lowering=False, debug=False,
                   num_devices=NCORES)

    def din(name, shape, dt):
        return nc.dram_tensor(name, shape, dt, kind="ExternalInput")

    xcol_d = din("xcol", [NPAIR, 128, 512], DT.bfloat16)
    w1T_d = din("w1T", [128, DIM], DT.bfloat16)
    r1T_d = din("r1T", [128, R, 4, HALF], DT.bfloat16)
    w2T_d = din("w2T", [128, R, 9, 2, HALF], DT.bfloat16)
    r3T_d = din("r3T", [128, R, 2, DIM], DT.bfloat16)
    encb_d = din("encb", [128, 4], DT.float32)
    b1_d = din("b1", [128, R, 2], DT.float32)
    b2_d = din("b2", [128, R, 2], DT.float32)
    b3_d = din("b3", [128, R, 4], DT.float32)
    gihT_d = din("gihT", [128, 4, 3 * HID], DT.bfloat16)
    ghhT_d = din("ghhT", [128, 2, 3 * HID], DT.bfloat16)
    gbih_d = din("gbih", [NB, 3 * HID], DT.float32)   # host-broadcast over batch
    gbhh_d = din("gbhh", [NB, 3 * HID], DT.float32)
    gbc_d = din("gbc", [NB, 2 * HID], DT.float32)     # (b_ih + b_hh)[: 512]
    wkT_d = din("wkT", [128, K, 2, DIM], DT.bfloat16)
    wkb_d = din("wkb", [128, K, 4], DT.float32)
    mask_d = din("mask", [NB, K * B], DT.float32)     # one-hot diag per (i,k)
    iota_d = din("iota64", [NB, B], DT.float32)       # 0..63 per row
    ident_d = din("ident8", [NB, NB], DT.float32)

    out_d = nc.dram_tensor("out", [NB, 2 * K], DT.float32, kind="ExternalOutput")

    # collective bounce buffers (internal DRAM, partition-major for cheap DMA)
    zin_b = nc.dram_tensor("zin_b", [128, 4, K * NB], DT.bfloat16)
    zout_b = nc.dram_tensor("zout_b", [NCORES, 128, 4, K * NB], DT.bfloat16,
                            addr_space="Shared")

    from contextlib import ExitStack
    with tile.TileContext(nc) as tc, ExitStack() as stack:
        wp = stack.enter_context(tc.tile_pool(name="weights", bufs=1))
        persist = stack.enter_context(tc.tile_pool(name="persist", bufs=1))
        y1p_pool = stack.enter_context(tc.tile_pool(name="y1p", bufs=2))
        xcp = stack.enter_context(tc.tile_pool(name="xc", bufs=4))
        hp = stack.enter_context(tc.tile_pool(name="h", bufs=6))
        y2p = stack.enter_context(tc.tile_pool(name="y2", bufs=3))
        psp = stack.enter_context(tc.tile_pool(name="psum", bufs=8, space="PSUM"))
        sp = stack.enter_context(tc.tile_pool(name="small", bufs=2))

        def wtile(dram, shape, dt, split_dim1=False):
            t = wp.tile(shape, dt, tag=dram.name, name=f"w_{dram.name}")
            if split_dim1:
                for i in range(shape[1]):
                    nc.scalar.dma_start(t[:, i], dram[:, i])
            else:
                nc.scalar.dma_start(t[:], dram[:])
            return t

        # conv weights first (needed by pair 0); w2T split per-resblock
        w1T = wtile(w1T_d, [128, DIM], DT.bfloat16)
        encb = wtile(encb_d, [128, 4], DT.float32)
        r1T = wtile(r1T_d, [128, R, 4, HALF], DT.bfloat16)
        b1 = wtile(b1_d, [128, R, 2], DT.float32)
        w2T = wtile(w2T_d, [128, R, 9, 2, HALF], DT.bfloat16, split_dim1=True)
        b2 = wtile(b2_d, [128, R, 2], DT.float32)
        r3T = wtile(r3T_d, [128, R, 2, DIM], DT.bfloat16)
        b3 = wtile(b3_d, [128, R, 4], DT.float32)
        # gru / loss weights (needed only after row t=0)
        gihT = wtile(gihT_d, [128, 4, 3 * HID], DT.bfloat16)
        ghhT = wtile(ghhT_d, [128, 2, 3 * HID], DT.bfloat16)
        gbih = wtile(gbih_d, [NB, 3 * HID], DT.float32)
        gbhh = wtile(gbhh_d, [NB, 3 * HID], DT.float32)
        gbc = wtile(gbc_d, [NB, 2 * HID], DT.float32)
        wkT = wtile(wkT_d, [128, K, 2, DIM], DT.bfloat16)
        wkb = wtile(wkb_d, [128, K, 4], DT.float32)
        mask = wtile(mask_d, [NB, K * B], DT.float32)
        iota64 = wtile(iota_d, [NB, B], DT.float32)
        ident = wtile(ident_d, [NB, NB], DT.float32)

        zbuf = persist.tile([128, 4, NIMG], DT.bfloat16, tag="zbuf")
        hT = persist.tile([128, 2, NB], DT.bfloat16, tag="hT")
        y40 = persist.tile([NB, K * B], DT.float32, tag="y40")
        out_sb = persist.tile([NB, 2 * K], DT.float32, tag="out_sb")
        y1pads = [y1p_pool.tile([128, 2, 2, 18, 18], DT.bfloat16, tag="y1pad",
                                name=f"y1pad{i}") for i in range(2)]
        for ypad in y1pads:
            nc.vector.memset(ypad[:], 0.0)

        # ---------------- conv encoder: one image pair ----------------
        def emit_pair(p):
            xc = xcp.tile([128, 512], DT.bfloat16, tag="xc", name=f"xc{p}")
            nc.sync.dma_start(xc[:], xcol_d[p])

            h = hp.tile([128, 4, 512], DT.bfloat16, tag="h", name=f"h{p}")
            for m in range(4):
                ps = psp.tile([128, 512], DT.float32, tag="ps", name=f"c1ps{p}{m}")
                nc.tensor.matmul(ps[:], w1T[:, m * 128:(m + 1) * 128], xc[:],
                                 start=True, stop=True)
                nc.scalar.activation(h[:, m], ps[:], ACTF.Relu,
                                     bias=encb[:, m:m + 1])

            for r in range(R):
                y1p = y1pads[p % 2]
                for m in range(2):
                    ps = psp.tile([128, 512], DT.float32, tag="ps",
                                  name=f"a_ps{p}{r}{m}")
                    for c in range(4):
                        nc.tensor.matmul(ps[:], r1T[:, r, c, m * 128:(m + 1) * 128],
                                         h[:, c], start=(c == 0), stop=(c == 3))
                    nc.scalar.activation(y1p[:, m, :, 1:17, 1:17],
                                         ps[:].rearrange("p (i r c) -> p i r c",
                                                         i=2, r=16),
                                         ACTF.Relu, bias=b1[:, r, m:m + 1])
                y2 = y2p.tile([128, 2, 512], DT.bfloat16, tag="y2", name=f"y2_{p}{r}")
                for m in range(2):
                    ps = psp.tile([128, 512], DT.float32, tag="ps",
                                  name=f"b_ps{p}{r}{m}")
                    i_mm = 0
                    for c in range(2):
                        for tap in range(9):
                            ky, kx = divmod(tap, 3)
                            rhs = y1p[:, c, :, ky:ky + 16, kx:kx + 16]
                            nc.tensor.matmul(
                                ps[:].rearrange("p (i r c) -> p i r c", i=2, r=16),
                                w2T[:, r, tap, c, m * 128:(m + 1) * 128],
                                rhs, start=(i_mm == 0), stop=(i_mm == 17))
                            i_mm += 1
                    nc.scalar.activation(y2[:, m], ps[:], ACTF.Relu,
                                         bias=b2[:, r, m:m + 1])
                hn = hp.tile([128, 4, 512], DT.bfloat16, tag="h", name=f"hn{p}{r}")
                ps3 = [psp.tile([128, 512], DT.float32, tag="ps",
                                name=f"c_ps{p}{r}{m}") for m in range(4)]
                for c in range(2):
                    for m in range(4):
                        nc.tensor.matmul(ps3[m][:],
                                         r3T[:, r, c, m * 128:(m + 1) * 128],
                                         y2[:, c], start=(c == 0), stop=(c == 1))
                for m in range(4):
                    nc.vector.tensor_add(hn[:, m], ps3[m][:], h[:, m])
                    nc.vector.tensor_scalar(hn[:, m], hn[:, m],
                                            b3[:, r, m:m + 1], 0.0,
                                            ALU.add, ALU.max)
                h = hn

            t_idx, j_idx = divmod(p, 4)
            col = t_idx * 8 + 2 * j_idx
            zt = sp.tile([128, 4, 2], DT.float32, tag="zt", name=f"zt{p}")
            nc.vector.tensor_reduce(zt[:], h[:].rearrange("p c (i x) -> p c i x",
                                                          i=2),
                                    mybir.AxisListType.X, ALU.add)
            nc.vector.tensor_scalar_mul(zbuf[:, :, col:col + 2], zt[:],
                                        1.0 / NPIX)

        # ---------------- GRU step (emitted after row t) ----------------
        gru_state = {'h': None}

        def emit_gru_step(t):
            gi_rz = psp.tile([NB, 2 * HID], DT.float32, tag="ps", name=f"girz{t}")
            gi_n = psp.tile([NB, HID], DT.float32, tag="ps", name=f"gin{t}")
            for c in range(4):
                nc.tensor.matmul(gi_rz[:], zbuf[:, c, t * 8:(t + 1) * 8],
                                 gihT[:, c, :2 * HID],
                                 start=(c == 0), stop=(c == 3))
            for c in range(4):
                nc.tensor.matmul(gi_n[:], zbuf[:, c, t * 8:(t + 1) * 8],
                                 gihT[:, c, 2 * HID:],
                                 start=(c == 0), stop=(c == 3))
            gh_rz = gh_n = None
            if t > 0:
                gh_rz = psp.tile([NB, 2 * HID], DT.float32, tag="ps",
                                 name=f"ghrz{t}")
                gh_n = psp.tile([NB, HID], DT.float32, tag="ps", name=f"ghn{t}")
                for c in range(2):
                    nc.tensor.matmul(gh_rz[:], hT[:, c, :], ghhT[:, c, :2 * HID],
                                     start=(c == 0), stop=(c == 1))
                for c in range(2):
                    nc.tensor.matmul(gh_n[:], hT[:, c, :], ghhT[:, c, 2 * HID:],
                                     start=(c == 0), stop=(c == 1))

            rz = sp.tile([NB, 2 * HID], DT.float32, tag="rz", name=f"rz{t}")
            ng = sp.tile([NB, HID], DT.float32, tag="ng", name=f"ng{t}")
            tmp = sp.tile([NB, HID], DT.float32, tag="gtmp", name=f"gtmp{t}")
            # r,z = sigmoid(gi_rz + gh_rz + (b_ih + b_hh)[:512])
            nc.vector.tensor_add(rz[:], gi_rz[:], gbc[:])
            if t > 0:
                nc.vector.tensor_add(rz[:], rz[:], gh_rz[:])
            nc.scalar.activation(rz[:], rz[:], ACTF.Sigmoid)
            # n = tanh(gi_n + b_ih_n + r * (gh_n + b_hh_n))
            if t > 0:
                nc.vector.tensor_add(tmp[:], gh_n[:], gbhh[:, 2 * HID:])
            else:
                nc.vector.tensor_copy(tmp[:], gbhh[:, 2 * HID:])
            nc.vector.tensor_mul(tmp[:], tmp[:], rz[:, :HID])
            nc.vector.tensor_add(ng[:], gi_n[:], gbih[:, 2 * HID:])
            nc.vector.tensor_add(ng[:], ng[:], tmp[:])
            nc.scalar.activation(ng[:], ng[:], ACTF.Tanh)
            # h = (1-z)*n + z*h_prev
            h_new = sp.tile([NB, HID], DT.float32, tag=f"hstep{t}",
                            name=f"hnew{t}")
            if t == 0:
                nc.vector.tensor_mul(tmp[:], rz[:, HID:], ng[:])
                nc.vector.tensor_sub(h_new[:], ng[:], tmp[:])
            else:
                nc.vector.tensor_sub(tmp[:], gru_state['h'][:], ng[:])
                nc.vector.tensor_mul(tmp[:], rz[:, HID:], tmp[:])
                nc.vector.tensor_add(h_new[:], ng[:], tmp[:])
            gru_state['h'] = h_new

        def emit_transposes(t):
            h_new = gru_state['h']
            for c in range(2):
                pt = psp.tile([128, NB], DT.float32, tag="ps", name=f"pt{t}{c}")
                nc.tensor.transpose(pt[:], h_new[:, c * 128:(c + 1) * 128],
                                    ident[:])
                nc.vector.tensor_copy(hT[:, c, :], pt[:])

        # -------- emit: conv rows with GRU / collective interleaved --------
        # after_pair[j] = ops to emit right after pair j of the NEXT row, so
        # GRU work hides under conv instead of stalling the PE queue.
        after_pair = {0: [], 1: []}
        ztk = persist.tile([128, NCORES, 4, K * NB], DT.bfloat16, tag="ztk")
        for t in ROWS:
            for j in range(4):
                emit_pair(t * 4 + j)
                if j in after_pair and after_pair[j]:
                    for fn in after_pair[j]:
                        fn()
                    after_pair[j] = []
            if t == 8:
                # ztk rows complete -> AllGather (gpsimd queue, overlaps conv)
                nc.gpsimd.dma_start(zin_b.ap(), zbuf[:, :, TCTX * 8:])
                nc.gpsimd.collective_compute(
                    "AllGather", ALU.bypass,
                    replica_groups=[list(range(NCORES))],
                    ins=[zin_b.ap().opt()], outs=[zout_b.ap().opt()])
                for core in range(NCORES):
                    nc.gpsimd.dma_start(ztk[:, core], zout_b[core])
            if t < 3:
                after_pair[0].append(lambda t=t: emit_gru_step(t))
                after_pair[1].append(lambda t=t: emit_transposes(t))
            elif t == 3:
                emit_gru_step(3)
                emit_transposes(3)

        # ---------------- preds + scores + loss ----------------
        preds = persist.tile([128, 4, K * NB], DT.bfloat16, tag="preds")
        for k in range(K):
            for m in range(4):
                pp = psp.tile([128, NB], DT.float32, tag="ps", name=f"pp{k}{m}")
                for c in range(2):
                    nc.tensor.matmul(pp[:], wkT[:, k, c, m * 128:(m + 1) * 128],
                                     hT[:, c, :], start=(c == 0), stop=(c == 1))
                nc.scalar.activation(preds[:, m, k * NB:(k + 1) * NB], pp[:],
                                     ACTF.Identity, bias=wkb[:, k, m:m + 1])

        for k in range(K):
            psk = psp.tile([NB, B], DT.float32, tag="ps", name=f"sck{k}")
            for c in range(4):
                nc.tensor.matmul(psk[:], preds[:, c, k * NB:(k + 1) * NB],
                                 ztk[:, :, c, k * NB:(k + 1) * NB],
                                 start=(c == 0), stop=(c == 3))
            nc.scalar.activation(y40[:, k * B:(k + 1) * B], psk[:], ACTF.Exp)

        for k in range(K):
            yk = y40[:, k * B:(k + 1) * B]
            mk = sp.tile([NB, 1], DT.float32, tag="mk", name=f"mk{k}")
            nmk = sp.tile([NB, 1], DT.float32, tag="nmk", name=f"nmk{k}")
            ek = sp.tile([NB, B], DT.float32, tag="ek", name=f"ek{k}")
            sek = sp.tile([NB, 1], DT.float32, tag="sek", name=f"sek{k}")
            lgk = sp.tile([NB, 1], DT.float32, tag="lgk", name=f"lgk{k}")
            dgk = sp.tile([NB, 1], DT.float32, tag="dgk", name=f"dgk{k}")
            eq = sp.tile([NB, B], DT.float32, tag="eq", name=f"eq{k}")
            nc.vector.tensor_reduce(mk[:], yk, mybir.AxisListType.X, ALU.max)
            nc.vector.tensor_scalar_mul(nmk[:], mk[:], -1.0)
            nc.scalar.activation(ek[:], yk, ACTF.Exp, bias=nmk[:])
            nc.vector.tensor_reduce(sek[:], ek[:], mybir.AxisListType.X, ALU.add)
            nc.scalar.activation(lgk[:], sek[:], ACTF.Ln)
            nc.vector.tensor_add(lgk[:], lgk[:], mk[:])     # lse
            nc.vector.tensor_mul(ek[:], yk, mask[:, k * B:(k + 1) * B])
            nc.vector.tensor_reduce(dgk[:], ek[:], mybir.AxisListType.X, ALU.add)
            nc.vector.tensor_sub(out_sb[:, k:k + 1], dgk[:], lgk[:])
            # argmax: onehot(y == max) dotted with iota
            nc.vector.tensor_scalar(eq[:], yk, mk[:], 0.0,
                                    ALU.subtract, ALU.is_equal)
            nc.vector.tensor_mul(eq[:], eq[:], iota64[:])
            nc.vector.tensor_reduce(out_sb[:, K + k:K + k + 1], eq[:],
                                    mybir.AxisListType.X, ALU.add)

        nc.sync.dma_start(out_d[:], out_sb[:])

    nc.compile()
    return nc


def host_prep(inputs):
    """Host-side prep: im2col for conv1, weight layout transforms, bf16 casts."""
    x = np.asarray(inputs['x'], F32)
    xp = np.pad(x, ((0, 0), (0, 0), (0, 0), (2, 2), (2, 2)))
    s = xp.strides
    xs = np.lib.stride_tricks.as_strided(
        xp, shape=(B, T, C, 5, 5, 16, 16),
        strides=(s[0], s[1], s[2], s[3], s[4], 2 * s[3], 2 * s[4]))
    x_col = np.ascontiguousarray(xs).reshape(B, T, 75, NPIX).astype(BF16)

    xcols = []
    for core in range(NCORES):
        xc = x_col[core * NB:(core + 1) * NB]
        arr = np.zeros((NPAIR, 128, 2 * NPIX), BF16)
        for t in range(T):
            for j in range(NB // 2):
                p = t * 4 + j
                arr[p, :75, :NPIX] = xc[2 * j, t]
                arr[p, :75, NPIX:] = xc[2 * j + 1, t]
        xcols.append(arr)

    w = {}
    w1T = np.zeros((128, DIM), BF16)
    w1T[:75] = np.asarray(inputs['enc_w'], F32).reshape(DIM, 75).T.astype(BF16)
    w['w1T'] = w1T
    r1 = np.asarray(inputs['res_w1'], F32).reshape(R, HALF, DIM).transpose(0, 2, 1)
    w['r1T'] = np.ascontiguousarray(
        r1.reshape(R, 4, 128, HALF).transpose(2, 0, 1, 3)).astype(BF16)
    r2 = np.asarray(inputs['res_w2'], F32).transpose(0, 3, 4, 2, 1)
    w['w2T'] = np.ascontiguousarray(
        r2.reshape(R, 9, 2, 128, HALF).transpose(3, 0, 1, 2, 4)).astype(BF16)
    r3 = np.asarray(inputs['res_w3'], F32).reshape(R, DIM, HALF).transpose(0, 2, 1)
    w['r3T'] = np.ascontiguousarray(
        r3.reshape(R, 2, 128, DIM).transpose(2, 0, 1, 3)).astype(BF16)
    w['encb'] = np.ascontiguousarray(
        np.asarray(inputs['enc_b'], F32).reshape(4, 128).T)
    w['b1'] = np.ascontiguousarray(
        np.asarray(inputs['res_b1'], F32).reshape(R, 2, 128).transpose(2, 0, 1))
    w['b2'] = np.ascontiguousarray(
        np.asarray(inputs['res_b2'], F32).reshape(R, 2, 128).transpose(2, 0, 1))
    w['b3'] = np.ascontiguousarray(
        np.asarray(inputs['res_b3'], F32).reshape(R, 4, 128).transpose(2, 0, 1))
    w['gihT'] = np.ascontiguousarray(
        np.asarray(inputs['gru_w_ih'], F32).T.reshape(4, 128, 3 * HID)
        .transpose(1, 0, 2)).astype(BF16)
    w['ghhT'] = np.ascontiguousarray(
        np.asarray(inputs['gru_w_hh'], F32).T.reshape(2, 128, 3 * HID)
        .transpose(1, 0, 2)).astype(BF16)
    bih = np.asarray(inputs['gru_b_ih'], F32)
    bhh = np.asarray(inputs['gru_b_hh'], F32)
    w['gbih'] = np.tile(bih[None, :], (NB, 1))
    w['gbhh'] = np.tile(bhh[None, :], (NB, 1))
    w['gbc'] = np.tile((bih + bhh)[None, :2 * HID], (NB, 1))
    wk = np.asarray(inputs['wk_w'], F32).transpose(0, 2, 1)
    w['wkT'] = np.ascontiguousarray(
        wk.reshape(K, 2, 128, DIM).transpose(2, 0, 1, 3)).astype(BF16)
    w['wkb'] = np.ascontiguousarray(
        np.asarray(inputs['wk_b'], F32).reshape(K, 4, 128).transpose(2, 0, 1))
    w['ident8'] = np.eye(NB, dtype=F32)
    w['iota64'] = np.tile(np.arange(B, dtype=F32)[None, :], (NB, 1))
    return xcols, w


_NC_CACHE = {}


def get_nc():
    if 'nc' not in _NC_CACHE:
        _NC_CACHE['nc'] = build_kernel()
    return _NC_CACHE['nc']


def make_in_maps(inputs):
    xcols, w = host_prep(inputs)
    in_maps = []
    for core in range(NCORES):
        m = dict(w)
        m['xcol'] = xcols[core]
        msk = np.zeros((NB, K, B), F32)
        for i in range(NB):
            msk[i, :, core * NB + i] = 1.0
        m['mask'] = msk.reshape(NB, K * B)
        in_maps.append(m)
    return in_maps


def reduce_outputs(results):
    tot, correct = 0.0, 0
    for core in range(NCORES):
        o = np.asarray(results[core]['out'], F32)   # [8, 10]
        tot += float(o[:, :K].sum())
        for i in range(NB):
            correct += int((o[i, K:] == core * NB + i).sum())
    loss = np.float32(-tot / (B * K))
    acc = np.float32(correct / (B * K))
    return loss, acc


def _install_ntff_hook():
    """Provide antenv.axon_hooks (missing in this image) so trace=True works."""
    try:
        from antenv.axon_hooks import get_axon_ntff_profile_hook  # noqa: F401
        return
    except ImportError:
        pass
    import ctypes
    import types
    import contextlib

    so_path = "/opt/axon/libaxon_pjrt.so"
    if not os.path.exists(so_path):
        return
    lib = ctypes.CDLL(so_path)
    if not hasattr(lib, "axon_start_nrt_profile"):
        return
    lib.axon_start_nrt_profile.argtypes = [ctypes.POINTER(ctypes.c_int64),
                                           ctypes.c_size_t]
    lib.axon_start_nrt_profile.restype = ctypes.c_int64
    lib.axon_stop_nrt_profile.argtypes = [ctypes.c_char_p]
    lib.axon_stop_nrt_profile.restype = ctypes.c_int64

    @contextlib.contextmanager
    def _hook(output_dir, device_ids):
        import jax
        jax.devices()
        if device_ids:
            ids = (ctypes.c_int64 * len(device_ids))(*device_ids)
            rc = lib.axon_start_nrt_profile(ids, len(device_ids))
        else:
            rc = lib.axon_start_nrt_profile(None, 0)
        if rc != 0:
            raise RuntimeError(f"axon_start_nrt_profile rc={rc}")
        try:
            yield
        finally:
            n = lib.axon_stop_nrt_profile(str(output_dir).encode())
            print(f"ntff profile: {n} file(s) written to {output_dir}")

    mod = types.ModuleType("antenv.axon_hooks")
    mod.get_axon_ntff_profile_hook = lambda: _hook
    mod.set_axon_ntff_profile_hook = lambda h: None
    import antenv
    antenv.axon_hooks = mod
    sys.modules["antenv.axon_hooks"] = mod


def run(inputs, trace=False, **kw):
    if trace:
        _install_ntff_hook()
    nc = get_nc()
    in_maps = make_in_maps(inputs)
    res = run_bass_kernel_spmd(nc, in_maps, core_ids=list(range(NCORES)),
                               trace=trace, **kw)
    return res


def kernel(**inputs):
    res = run(inputs, trace=False)
    return reduce_outputs(res.results)


if __name__ == '__main__':
    import reference as Rf
    inputs = {k: np.asarray(v) for k, v in Rf.setup_inputs().items()}
    loss, acc = kernel(**inputs)
    print('kernel loss/acc:', loss, acc)
